# revision 48
# baseline (speedup 1.0000x reference)
"""CategoryAwareDAHEAD Trainium2 kernel (8-core SPMD, data-parallel over ROIs).

v2 strategy (roofline: ~21.4MB HBM/core => ~60us stream + short tail)
---------------------------------------------------------------------
* ins_features cast to bf16 HOST-side and sharded 64 rows/core; streamed in
  4 DMAs of 3.2MB on the sync (SP) HWDGE ring, pooled 7x7 by DVE reduces
  into opT [128, 16*64] f32 (feature f = 128*g + p, a fixed permutation of
  original d; W1's rows are permuted identically host-side).
* W1/W2/W3/Wd cast to bf16 host-side, queued on the SAME sync ring right
  AFTER the features (FIFO per HWDGE ring => features stream first, weights
  arrive exactly when the EA matmuls need them). All weight tiles persist
  in SBUF. EA chain (3x matmul+LN+transpose) runs bf16 with f32 PSUM.
* The sequential threshold-gated EMA scan is restructured as in v1:
  th <- max(th, (th+m)/2) run per-class on [21, 2S] tables built with
  onehot/prefix-sum matmuls; per-sample weights w = 1/(j*...*k) via
  exp(-suffix_sum(A*ln(J))) matmuls. All from f32 logits (exact gating).
* Cross-core reduction is SPLIT:
  - MGRM partial prototypes are produced TRANSPOSED ([2048 d-rows, 64]:
    cols = [s_protos | t_protos | bwl_sums | pad]) and ReduceScatter'd, so
    each core owns a fully-reduced D-slice [256, 64]. Each core computes
    partial Gram dots (ptm/btm [21,21]) + norm partials on its slice.
  - One tiny AllReduce [64, 64] then carries dot partials + norm partials
    + the loss_ea partial. The EA branch overlaps the ReduceScatter.
  Cosines are recovered as dot * rn_row * rn_col (scale-invariant, the
  1/49 pooling scale and 1/count bwl scale fold into the rn factors).

KLEVEL env: full (default) | nocc (replace collectives with local copies)
"""

import os
import sys

for _p in ("/opt/trn_rl_repo", "/root/.axon_site/_ro/trn_rl_repo"):
    if _p not in sys.path:
        sys.path.insert(0, _p)

import numpy as np
import ml_dtypes

import concourse.bacc as bacc
import concourse.mybir as mybir
import concourse.tile as tile
from concourse import bass_utils
from concourse.masks import make_identity, make_upper_triangular, make_lower_triangular

F32 = mybir.dt.float32
BF16 = mybir.dt.bfloat16
I32 = mybir.dt.int32
AX = mybir.AxisListType
OP = mybir.AluOpType
ACT = mybir.ActivationFunctionType
BF = ml_dtypes.bfloat16

NCORES = 8
N, NS, C, D = 512, 256, 21, 2048
NL = N // NCORES          # 64 rows per core
H1 = 1024
DIN = D + C               # 2069
S = 32                    # per-class sequence table length (max count is 18)
GN = 8                    # samples per pooling DMA (8 DMAs x 1.6MB bf16)
THR0, MOM, EPS, LN_EPS, MGRM_W = 0.1, 0.5, 1e-8, 1e-5, 1.0
NT = D // 128             # 16 feature blocks

# feature permutation: new index f holds original d = (f % 128) * 16 + f // 128
PERM = (np.arange(D) % 128) * 16 + np.arange(D) // 128


def _build():
    lv = os.environ.get("KLEVEL", "full")
    use_cc = lv != "nocc"

    nc = bacc.Bacc("TRN2", target_bir_lowering=False, debug=False,
                   num_devices=NCORES)

    # ---------------- DRAM I/O ----------------
    # features host-padded 7x7=49 -> 50 per (n, d) so every PE pooling
    # operand has even element offsets/strides (2-byte dtype)
    feat_dr = nc.dram_tensor("feat_shard", [NL, D, 50], BF16, kind="ExternalInput")
    lg_dr = nc.dram_tensor("logits_full", [N, C], F32, kind="ExternalInput")
    lgT_dr = nc.dram_tensor("logitsT_loc", [C, NL], BF16, kind="ExternalInput")
    lab_dr = nc.dram_tensor("labels_in", [NS], I32, kind="ExternalInput")
    dom_dr = nc.dram_tensor("dom_shard", [NL, 1], I32, kind="ExternalInput")
    sels_dr = nc.dram_tensor("sel_src", [NS, NL], F32, kind="ExternalInput")
    selt_dr = nc.dram_tensor("sel_tgt", [NS, NL], F32, kind="ExternalInput")
    w1_dr = nc.dram_tensor("W1p", [DIN, H1], BF16, kind="ExternalInput")
    w2_dr = nc.dram_tensor("W2in", [H1, H1], BF16, kind="ExternalInput")
    w3_dr = nc.dram_tensor("W3in", [H1, H1], BF16, kind="ExternalInput")
    wdb_dr = nc.dram_tensor("Wdb", [NL, H1], BF16, kind="ExternalInput")
    b1_dr = nc.dram_tensor("b1in", [1, H1], BF16, kind="ExternalInput")
    b2_dr = nc.dram_tensor("b2in", [1, H1], BF16, kind="ExternalInput")
    b3_dr = nc.dram_tensor("b3in", [1, H1], BF16, kind="ExternalInput")
    bd_dr = nc.dram_tensor("bdin", [NL, 1], F32, kind="ExternalInput")
    out_dr = nc.dram_tensor("out_loss", [1, 2], F32, kind="ExternalOutput")

    with tile.TileContext(nc) as tc:
        with (
            tc.tile_pool(name="consts", bufs=1) as cst,
            tc.tile_pool(name="insb", bufs=1) as insb,
            tc.tile_pool(name="featp", bufs=4) as featp,
            tc.tile_pool(name="persist", bufs=1) as per,
            tc.tile_pool(name="wpool", bufs=1) as wp,
            tc.tile_pool(name="work", bufs=1) as wk,
            tc.tile_pool(name="pps", bufs=2, space="PSUM") as pps,
            tc.tile_pool(name="ppool", bufs=4, space="PSUM") as ppool,
            tc.tile_pool(name="pph", bufs=1, space="PSUM") as pph,
            tc.tile_pool(name="dram", bufs=1, space="DRAM") as drp,
        ):
            # ---------------- constants ----------------
            id128 = cst.tile([128, 128], F32, tag="id128")
            make_identity(nc, id128[:])
            id128b = cst.tile([128, 128], BF16, tag="id128b")
            make_identity(nc, id128b[:])
            ut128 = cst.tile([128, 128], F32, tag="ut128")
            make_upper_triangular(nc, ut128[:], val=1.0, diag=True)
            lt128 = cst.tile([128, 128], F32, tag="lt128")
            make_lower_triangular(nc, lt128[:], val=1.0, diag=True)
            ones = cst.tile([128, 128], F32, tag="ones")
            nc.gpsimd.memset(ones[:], 1.0)
            iotaS = cst.tile([128, S], F32, tag="iotaS")   # 1..S per partition
            nc.gpsimd.iota(iotaS[:], [[1, S]], base=1, channel_multiplier=0,
                           allow_small_or_imprecise_dtypes=True)
            iota21 = cst.tile([128, C], F32, tag="iota21")  # 0..20
            nc.gpsimd.iota(iota21[:], [[1, C]], base=0, channel_multiplier=0,
                           allow_small_or_imprecise_dtypes=True)
            epsln = cst.tile([128, 1], F32, tag="epsln")
            nc.gpsimd.memset(epsln[:], LN_EPS)
            onesb = cst.tile([1, 128], BF16, tag="onesb")
            nc.gpsimd.memset(onesb[:], 1.0)

            # ------------- small input DMAs (ACT HWDGE ring) -------------
            lg_sb = insb.tile([128, 4 * C], F32, tag="lg")       # [128, 84]
            nc.scalar.dma_start(
                lg_sb[:].rearrange("p (c l) -> p c l", c=4),
                lg_dr.ap().rearrange("(c p) l -> p c l", p=128))
            lgT_sb = insb.tile([C, NL], BF16, tag="lgT")
            nc.scalar.dma_start(lgT_sb[:], lgT_dr[:, :])
            lab_i = insb.tile([128, 2], I32, tag="labi")
            nc.scalar.dma_start(lab_i[:], lab_dr.ap().rearrange(
                "(c p) -> p c", p=128))
            dom_i = insb.tile([NL, 1], I32, tag="domi")
            nc.scalar.dma_start(dom_i[:], dom_dr[:, :])
            sels_sb = insb.tile([128, 2 * NL], F32, tag="sels")
            nc.scalar.dma_start(
                sels_sb[:].rearrange("p (c n) -> p c n", c=2),
                sels_dr.ap().rearrange("(c p) n -> p c n", p=128))
            selt_sb = insb.tile([128, 2 * NL], F32, tag="selt")
            nc.scalar.dma_start(
                selt_sb[:].rearrange("p (c n) -> p c n", c=2),
                selt_dr.ap().rearrange("(c p) n -> p c n", p=128))
            b1_sb = insb.tile([1, H1], BF16, tag="b1")
            nc.scalar.dma_start(b1_sb[:], b1_dr[:, :])
            b2_sb = insb.tile([1, H1], BF16, tag="b2")
            nc.scalar.dma_start(b2_sb[:], b2_dr[:, :])
            b3_sb = insb.tile([1, H1], BF16, tag="b3")
            nc.scalar.dma_start(b3_sb[:], b3_dr[:, :])
            bd_sb = insb.tile([NL, 1], F32, tag="bd")
            nc.scalar.dma_start(bd_sb[:], bd_dr[:, :])
            wdb_sb = insb.tile([NL, H1], BF16, tag="wdb")
            nc.scalar.dma_start(wdb_sb[:], wdb_dr[:, :])

            lab_f = insb.tile([128, 2], F32, tag="labf")
            nc.vector.tensor_copy(lab_f[:], lab_i[:])
            dom_f = insb.tile([NL, 1], F32, tag="domf")
            nc.vector.tensor_copy(dom_f[:], dom_i[:])

            # -------- feature stream + pooling (SP HWDGE ring, FIFO) --------
            # feat_dr [NL, 2048, 7, 7] bf16 viewed as [128(p), NL, 784] where
            # partition p holds original d in [16p, 16p+16), 784 = 16*49.
            # Pooling on the PE: 25 accumulating matmuls with a stationary
            # identity sum the (padded) 7x7 window two taps at a time at
            # 128 elem/cycle, freeing the DVE for the threshold-chain work
            # that runs concurrently. Even offsets/strides throughout.
            feat_ap = feat_dr.ap().rearrange("n (p g) w -> p n (g w)", p=128)
            opT = per.tile([128, 16 * NL], F32, tag="opT")   # RAW sums (x49)
            opT3 = opT[:].rearrange("p (g n) -> p g n", g=16)
            fts = []
            for n0 in range(0, NL, GN):
                ft = featp.tile([128, GN * 800], BF16, tag="ft")
                nc.sync.dma_start(ft[:], feat_ap[:, n0:n0 + GN, :])
                fts.append(ft)
            # (pooling matmuls are emitted AFTER the threshold-chain section
            # below: the PE runs its queue strictly in order, and the chain's
            # small matmuls must not sit behind 100 pooling matmuls)

            # -------- weight prefetch (queued behind features, same ring) ----
            w1_sb = []
            for kc in range(17):
                kp = 128 if kc < 16 else C
                wt = wp.tile([128, H1], BF16, tag=f"w1_{kc}")
                nc.sync.dma_start(wt[0:kp, :], w1_dr[128 * kc:128 * kc + kp, :])
                w1_sb.append(wt)
            w2_sb, w3_sb = [], []
            for w_dr, lst, nm in ((w2_dr, w2_sb, "w2"), (w3_dr, w3_sb, "w3")):
                for kc in range(8):
                    wt = wp.tile([128, H1], BF16, tag=f"{nm}_{kc}")
                    nc.sync.dma_start(wt[:], w_dr[128 * kc:128 * (kc + 1), :])
                    lst.append(wt)

            # ---------------- per-chunk softmax stats (f32, exact) ----------
            E_ch, OHP_ch, mlOHP_ch = [], [], []
            for ch in range(4):
                lg_c = lg_sb[:, C * ch:C * (ch + 1)]
                mx = wk.tile([128, 1], F32, tag=f"mx{ch}")
                nc.vector.tensor_reduce(mx[:], lg_c, axis=AX.X, op=OP.max)
                E = wk.tile([128, C], F32, tag=f"E{ch}")
                nc.vector.tensor_scalar(out=E[:], in0=lg_c, scalar1=mx[:],
                                        scalar2=None, op0=OP.is_equal)
                negmx = wk.tile([128, 1], F32, tag=f"nmx{ch}")
                nc.vector.tensor_scalar_mul(negmx[:], mx[:], -1.0)
                scr = wk.tile([128, C], F32, tag=f"scr{ch}")
                den = wk.tile([128, 1], F32, tag=f"den{ch}")
                nc.scalar.activation(scr[:], lg_c, ACT.Exp, bias=negmx[:],
                                     scale=1.0, accum_out=den[:])
                ml = wk.tile([128, 1], F32, tag=f"ml{ch}")
                nc.vector.reciprocal(ml[:], den[:])
                E_ch.append(E)

                dom = ch // 2   # 0 = src, 1 = tgt
                P_ps = pps.tile([128, C], F32, tag="pps")
                if ch % 2 == 0:
                    nc.tensor.matmul(P_ps[:], ut128[:], E[:], start=True,
                                     stop=True)
                else:
                    nc.tensor.matmul(P_ps[:], ones[:], E_ch[2 * dom][:],
                                     start=True, stop=False)
                    nc.tensor.matmul(P_ps[:], ut128[:], E[:], start=False,
                                     stop=True)
                pos = wk.tile([128, 1], F32, tag=f"pos{ch}")
                posscr = wk.tile([128, C], F32, tag=f"poss{ch}")
                nc.vector.tensor_tensor(posscr[:], P_ps[:], E[:], op=OP.mult)
                nc.vector.tensor_reduce(pos[:], posscr[:], axis=AX.X,
                                        op=OP.add)
                OHP = wk.tile([128, S], F32, tag=f"OHP{ch}")
                nc.vector.tensor_scalar(out=OHP[:], in0=iotaS[:],
                                        scalar1=pos[:], scalar2=None,
                                        op0=OP.is_equal)
                mlOHP = wk.tile([128, S], F32, tag=f"mlO{ch}")
                nc.vector.tensor_scalar(out=mlOHP[:], in0=iotaS[:],
                                        scalar1=pos[:], scalar2=ml[:],
                                        op0=OP.is_equal, op1=OP.mult)
                OHP_ch.append(OHP)
                mlOHP_ch.append(mlOHP)

            # tables T[21, 2S]: cols 0..S-1 src, S..2S-1 tgt (dom-major)
            T_all = wk.tile([C, 2 * S], F32, tag="Tall")
            for dom in range(2):
                T_ps = pps.tile([C, S], F32, tag="pps")
                nc.tensor.matmul(T_ps[:], E_ch[2 * dom][:],
                                 mlOHP_ch[2 * dom][:], start=True, stop=False)
                nc.tensor.matmul(T_ps[:], E_ch[2 * dom + 1][:],
                                 mlOHP_ch[2 * dom + 1][:], start=False,
                                 stop=True)
                nc.scalar.copy(T_all[:, S * dom:S * (dom + 1)], T_ps[:])

            # E-transposes for the EW gathers (depend only on E_ch: emit
            # early so they do not serialize the later EW section)
            ET_ch = []
            for ch in range(4):
                ET = wk.tile([C, 128], F32, tag=f"ET{ch}")
                et_ps = pps.tile([C, 128], F32, tag="pps")
                nc.tensor.transpose(et_ps[:], E_ch[ch][:], id128[:])
                nc.scalar.copy(ET[:], et_ps[:])
                ET_ch.append(ET)

            # pooling matmuls, interleaved into the chain's DVE-bound gaps
            # (the PE runs strictly in order; 25 accumulating matmuls per
            # group sum the padded 7x7 window two taps at a time)
            pool_pss = []

            def emit_pool_group(gi):
                ftj = fts[gi][:].rearrange("p (x j k) -> p j x k", j=25, k=2)
                pool_ps = ppool.tile([128, GN * 16 * 2], F32, tag="pool")
                pool2 = pool_ps[:].rearrange("p (x k) -> p x k", k=2)
                for j in range(25):
                    nc.tensor.matmul(pool2, id128b[:], ftj[:, j, :, :],
                                     start=(j == 0), stop=(j == 24))
                pool_pss.append(pool_ps)

            def emit_recombine(gi):
                rn0 = gi * GN
                rp2 = pool_pss[gi][:].rearrange("p (x k) -> p x k", k=2)
                nc.vector.tensor_reduce(
                    opT3[:, :, rn0:rn0 + GN].rearrange("p g n -> p n g"),
                    rp2, axis=AX.X, op=OP.add)

            for _g in (0, 1, 2):
                emit_pool_group(_g)

            # ------------- sequential threshold chain -------------
            # 2 DVE ops per step on a threshold-history buffer, then ONE
            # batched is_ge produces all accept bits:
            #   thh[s+1] = max(thh[s], 0.5*thh[s] + 0.5*m_s);  A_s = m_s>=thh[s]
            T3 = T_all[:].rearrange("p (d s) -> p d s", d=2)
            thh = wk.tile([C, 2 * (S + 1)], F32, tag="thh")
            nc.gpsimd.memset(thh[:], THR0)
            thh3 = thh[:].rearrange("p (d s) -> p d s", d=2)
            halfm = wk.tile([C, 2 * S], F32, tag="halfm")
            nc.vector.tensor_scalar_mul(halfm[:], T_all[:], 0.5)
            halfm3 = halfm[:].rearrange("p (d s) -> p d s", d=2)
            tmp = wk.tile([C, 2], F32, tag="chtmp")
            for s in range(S):
                nc.vector.scalar_tensor_tensor(
                    out=tmp[:], in0=thh3[:, :, s], scalar=0.5,
                    in1=halfm3[:, :, s], op0=OP.mult, op1=OP.add)
                nc.vector.tensor_tensor(thh3[:, :, s + 1], tmp[:],
                                        thh3[:, :, s], op=OP.max)
            A = wk.tile([C, 2 * S], F32, tag="A")
            A3 = A[:].rearrange("p (d s) -> p d s", d=2)
            nc.vector.tensor_tensor(A3[:, :, :], T3[:, :, :],
                                    thh3[:, :, 0:S], op=OP.is_ge)

            for _g in (0, 1, 2):
                emit_recombine(_g)
            for _g in (3, 4):
                emit_pool_group(_g)

            # per-domain accept-index math -> wtab [21, 2S]
            wtab = wk.tile([C, 2 * S], F32, tag="wtab")
            for dom in range(2):
                A_dom = A[:, S * dom:S * (dom + 1)]
                A_T = wk.tile([S, C], F32, tag=f"AT{dom}")
                at_ps = pps.tile([S, C], F32, tag="pps")
                nc.tensor.transpose(at_ps[:], A_dom, id128[0:C, 0:C])
                nc.scalar.copy(A_T[:], at_ps[:])
                J_ps = pps.tile([C, S], F32, tag="pps")
                nc.tensor.matmul(J_ps[:], A_T[:], ut128[0:S, 0:S],
                                 start=True, stop=True)
                jc = wk.tile([C, S], F32, tag=f"jc{dom}")
                nc.vector.tensor_scalar(out=jc[:], in0=J_ps[:], scalar1=1.0,
                                        scalar2=None, op0=OP.max)
                lnJ = wk.tile([C, S], F32, tag=f"lnJ{dom}")
                nc.scalar.activation(lnJ[:], jc[:], ACT.Ln)
                nc.vector.tensor_tensor(lnJ[:], lnJ[:], A_dom, op=OP.mult)
                lnJ_T = wk.tile([S, C], F32, tag=f"lnJT{dom}")
                lt_ps = pps.tile([S, C], F32, tag="pps")
                nc.tensor.transpose(lt_ps[:], lnJ[:], id128[0:C, 0:C])
                nc.scalar.copy(lnJ_T[:], lt_ps[:])
                SS_ps = pps.tile([C, S], F32, tag="pps")
                nc.tensor.matmul(SS_ps[:], lnJ_T[:], lt128[0:S, 0:S],
                                 start=True, stop=True)
                wt_dom = wtab[:, S * dom:S * (dom + 1)]
                nc.scalar.activation(wt_dom, SS_ps[:], ACT.Exp, scale=-1.0)
                nc.vector.tensor_tensor(wt_dom, wt_dom, A_dom, op=OP.mult)

            for _g in (5, 6, 7):
                emit_pool_group(_g)

            # remaining recombines (emitted late so the DVE queue's chain
            # work is not stalled)
            for _g in (3, 4, 5, 6, 7):
                emit_recombine(_g)

            # -------- opT consumers: bf16 copy (EA) + feat64 (protos) -------
            opTb = per.tile([128, 16 * NL], BF16, tag="opTb")  # scaled 1/49
            nc.vector.tensor_scalar_mul(opTb[:], opT[:], 1.0 / 49.0)

            # feat64 [64, 2048] sample-major (bf16 PE transpose of opTb blocks;
            # the uniform 1/49 scale cancels in every cosine downstream)
            feat64 = per.tile([NL, D], BF16, tag="feat64")
            for half in range(2):
                tp = pps.tile([NL, 1024], BF16, tag="pps")
                for g in range(8):
                    gg = half * 8 + g
                    nc.tensor.transpose(tp[:, 128 * g:128 * (g + 1)],
                                        opTb[:, NL * gg:NL * (gg + 1)],
                                        id128b[:])
                nc.scalar.copy(feat64[:, 1024 * half:1024 * (half + 1)], tp[:])

            # preload the remaining ACT tables (4 slots) so the EA/BCE tail
            # and final combine pay no ACT_TABLE_LOADs on the critical path
            dumact = wk.tile([1, 1], F32, tag="dumact")
            nc.scalar.activation(dumact[:], epsln[0:1, :], ACT.Sigmoid)
            nc.scalar.activation(dumact[:], epsln[0:1, :], ACT.Sqrt)

            # ---------- per-sample weights, local gathers ----------
            # EWl_all [64, 64]: cols 0:21 src-proto w, 21:42 tgt-proto w,
            # 42:63 label onehot, 63 pad(0)
            EWl_all = wk.tile([NL, 64], BF16, tag="EWall")
            nc.gpsimd.memset(EWl_all[:], 0.0)
            for dom in range(2):
                sel = sels_sb if dom == 0 else selt_sb
                ewl_ps = pps.tile([NL, C], F32, tag="pps")
                for cc in range(2):
                    ch = 2 * dom + cc
                    G_ps = pps.tile([128, S], F32, tag="pps")
                    nc.tensor.matmul(G_ps[:], ET_ch[ch][:],
                                     wtab[:, S * dom:S * (dom + 1)],
                                     start=True, stop=True)
                    ws = wk.tile([128, 1], F32, tag=f"ws{ch}")
                    wscr = wk.tile([128, S], F32, tag=f"wscr{ch}")
                    nc.vector.tensor_tensor(wscr[:], G_ps[:], OHP_ch[ch][:],
                                            op=OP.mult)
                    nc.vector.tensor_reduce(ws[:], wscr[:], axis=AX.X,
                                            op=OP.add)
                    EW = wk.tile([128, C], F32, tag=f"EW{ch}")
                    nc.vector.tensor_scalar(out=EW[:], in0=E_ch[ch][:],
                                            scalar1=ws[:], scalar2=None,
                                            op0=OP.mult)
                    nc.tensor.matmul(ewl_ps[:], sel[:, NL * cc:NL * (cc + 1)],
                                     EW[:], start=(cc == 0), stop=(cc == 1))
                nc.scalar.copy(EWl_all[:, 21 * dom:21 * (dom + 1)], ewl_ps[:])

            # labels onehot + counts + local label gather
            elab_ps = pps.tile([NL, C], F32, tag="pps")
            cnt_ps = pps.tile([C, 1], F32, tag="pps")
            for cc in range(2):
                Elab = wk.tile([128, C], F32, tag=f"Elab{cc}")
                nc.vector.tensor_scalar(out=Elab[:], in0=iota21[:],
                                        scalar1=lab_f[:, cc:cc + 1],
                                        scalar2=None, op0=OP.is_equal)
                nc.tensor.matmul(cnt_ps[:], Elab[:], ones[:, 0:1],
                                 start=(cc == 0), stop=(cc == 1))
                nc.tensor.matmul(elab_ps[:], sels_sb[:, NL * cc:NL * (cc + 1)],
                                 Elab[:], start=(cc == 0), stop=(cc == 1))
            nc.scalar.copy(EWl_all[:, 42:63], elab_ps[:])
            counts = wk.tile([C, 1], F32, tag="counts")
            nc.scalar.copy(counts[:], cnt_ps[:])

            # count-derived scalars for the final combine (hoisted: DVE is
            # free here, and gpsimd memsets must precede the collective
            # waits that block the gpsimd queue)
            cexist = wk.tile([C, 1], F32, tag="cex")
            nc.vector.tensor_scalar(out=cexist[:], in0=counts[:], scalar1=0.0,
                                    scalar2=None, op0=OP.is_gt)
            cem0 = wk.tile([C, 1], F32, tag="cem0")
            nc.vector.tensor_copy(cem0[:], cexist[:])
            nc.gpsimd.memset(cem0[0:1, :], 0.0)
            rmask0 = wk.tile([C, 1], F32, tag="rmask0")
            nc.gpsimd.memset(rmask0[:], 1.0)
            nc.gpsimd.memset(rmask0[0:1, :], 0.0)
            cmax = wk.tile([C, 1], F32, tag="cmax")
            nc.vector.tensor_scalar(out=cmax[:], in0=counts[:], scalar1=1.0,
                                    scalar2=None, op0=OP.max)
            crec = wk.tile([C, 1], F32, tag="crec")
            nc.vector.reciprocal(crec[:], cmax[:])

            # ---- transposed partial protos prT [2048, 64] -> AllToAll ----
            # (A2A = pure copies on the fast mesh path; the 8-way shard sum
            # happens locally on DVE. Equivalent to ReduceScatter but ~4x
            # cheaper at this size on the 8-rank ring.)
            bounce1 = drp.tile([D, 64], BF16, tag="bounce1")
            bounce1_out = drp.tile([D, 64], BF16, tag="bounce1o")
            prT_ps = pph.tile([128, 16 * 64], F32, tag="pph")
            for t in range(NT):
                nc.tensor.matmul(prT_ps[:, 64 * t:64 * (t + 1)],
                                 feat64[:, 128 * t:128 * (t + 1)],
                                 EWl_all[:], start=True, stop=True)
            # payload tile for the tiny AllReduce (filled piecewise below);
            # memset BEFORE the RS trigger: gpsimd blocks on collective waits
            asm = wk.tile([64, 64], F32, tag="asm")
            nc.gpsimd.memset(asm[:], 0.0)

            prT_sb = per.tile([128, 16 * 64], BF16, tag="prTsb")
            nc.scalar.copy(prT_sb[:], prT_ps[:])
            nc.scalar.dma_start(
                bounce1[:].rearrange("(t p) c -> p t c", p=128),
                prT_sb[:].rearrange("p (t c) -> p t c", t=16))
            if use_cc:
                nc.gpsimd.collective_compute(
                    "AllToAll", OP.bypass,
                    replica_groups=[list(range(NCORES))],
                    ins=[bounce1[:].opt()], outs=[bounce1_out[:].opt()])
            else:
                nc.sync.dma_start(bounce1_out[:], bounce1[:])

            bounce2 = drp.tile([64, 64], F32, tag="bounce2")
            bounce2_out = drp.tile([8 * 64, 64], F32, tag="bounce2o")

            # ---------------- EA branch (bf16, per-core rows) ----------------
            def ln_relu(h_ps, li):
                s1 = wk.tile([NL, 1], F32, tag="s1")
                nc.vector.tensor_reduce(s1[:], h_ps[:], axis=AX.X, op=OP.add)
                s2scr = wk.tile([NL, H1], BF16, tag="s2scr")
                s2 = wk.tile([NL, 1], F32, tag="s2")
                nc.scalar.activation(s2scr[:], h_ps[:], ACT.Square,
                                     accum_out=s2[:])
                mu = wk.tile([NL, 1], F32, tag="mu")
                nc.vector.tensor_scalar_mul(mu[:], s1[:], 1.0 / H1)
                ms2 = wk.tile([NL, 1], F32, tag="ms2")
                nc.vector.tensor_scalar_mul(ms2[:], s2[:], 1.0 / H1)
                nvar = wk.tile([NL, 1], F32, tag="nvar")
                nc.vector.scalar_tensor_tensor(out=nvar[:], in0=mu[:],
                                               scalar=mu[:], in1=ms2[:],
                                               op0=OP.mult, op1=OP.subtract)
                sd = wk.tile([NL, 1], F32, tag="sd")
                nc.scalar.activation(sd[:], nvar[:], ACT.Sqrt, scale=-1.0,
                                     bias=epsln[0:NL, :])
                rstd = wk.tile([NL, 1], F32, tag="rstd")
                nc.vector.reciprocal(rstd[:], sd[:])
                mb = wk.tile([NL, 1], F32, tag="mb")
                nc.vector.tensor_scalar(out=mb[:], in0=mu[:], scalar1=rstd[:],
                                        scalar2=-1.0, op0=OP.mult, op1=OP.mult)
                h = wk.tile([NL, H1], BF16, tag=f"h{li}")
                nc.scalar.activation(h[:], h_ps[:], ACT.Relu, bias=mb[:],
                                     scale=rstd[:])
                return h

            def transpose_h2(h, li):
                hT = wk.tile([128, 8 * NL], BF16, tag=f"hT{li}")
                for half in range(2):
                    ht_ps = pps.tile([128, 4 * NL], BF16, tag="pps")
                    for j in range(4):
                        jj = 4 * half + j
                        nc.tensor.transpose(ht_ps[:, NL * j:NL * (j + 1)],
                                            h[:, 128 * jj:128 * (jj + 1)],
                                            id128b[0:NL, 0:NL])
                    nc.scalar.copy(hT[:, 4 * NL * half:4 * NL * (half + 1)],
                                   ht_ps[:])
                return hT

            # layer 1: lhsT chunks = opTb blocks + logitsT (all bf16)
            h_ps = pph.tile([NL, H1], F32, tag="pph")
            for half in range(2):
                nsl = slice(512 * half, 512 * (half + 1))
                nc.tensor.matmul(h_ps[:, nsl], onesb[:, 0:NL],
                                 b1_sb[:, nsl], start=True, stop=False)
            for kc in range(17):
                kp = 128 if kc < 16 else C
                lhsT = (opTb[:, NL * kc:NL * (kc + 1)] if kc < 16
                        else lgT_sb[:, :])
                for half in range(2):
                    nsl = slice(512 * half, 512 * (half + 1))
                    nc.tensor.matmul(h_ps[:, nsl], lhsT,
                                     w1_sb[kc][0:kp, nsl],
                                     start=False, stop=(kc == 16))
            h = ln_relu(h_ps, 1)

            for li, (wl, b_sb) in enumerate(((w2_sb, b2_sb), (w3_sb, b3_sb))):
                hT = transpose_h2(h, li + 1)
                h_ps = pph.tile([NL, H1], F32, tag="pph")
                for half in range(2):
                    nsl = slice(512 * half, 512 * (half + 1))
                    nc.tensor.matmul(h_ps[:, nsl], onesb[:, 0:NL],
                                     b_sb[:, nsl], start=True, stop=False)
                for kc in range(8):
                    for half in range(2):
                        nsl = slice(512 * half, 512 * (half + 1))
                        nc.tensor.matmul(h_ps[:, nsl],
                                         hT[:, NL * kc:NL * (kc + 1)],
                                         wl[kc][:, nsl],
                                         start=False, stop=(kc == 7))
                h = ln_relu(h_ps, li + 2)

            # domain head on DVE (avoids a third transpose round)
            zscr = wk.tile([NL, H1], F32, tag="zscr")
            nc.vector.tensor_tensor(zscr[:], h[:], wdb_sb[:], op=OP.mult)
            zd = wk.tile([NL, 1], F32, tag="zd")
            nc.vector.tensor_reduce(zd[:], zscr[:], axis=AX.X, op=OP.add)
            z = wk.tile([NL, 1], F32, tag="z")
            nc.scalar.activation(z[:], zd[:], ACT.Sigmoid, bias=bd_sb[:])
            # BCE-with-logits on z: li = z*(1-y) + ln(1 + exp(-z))  (z >= 0)
            enz = wk.tile([NL, 1], F32, tag="enz")
            nc.scalar.activation(enz[:], z[:], ACT.Exp, scale=-1.0)
            sp = wk.tile([NL, 1], F32, tag="sp")
            nc.scalar.activation(sp[:], enz[:], ACT.Ln, bias=ones[0:NL, 0:1])
            omy = wk.tile([NL, 1], F32, tag="omy")
            nc.vector.tensor_scalar(out=omy[:], in0=dom_f[:], scalar1=-1.0,
                                    scalar2=1.0, op0=OP.mult, op1=OP.add)
            li_t = wk.tile([NL, 1], F32, tag="li")
            nc.vector.scalar_tensor_tensor(out=li_t[:], in0=z[:],
                                           scalar=omy[:], in1=sp[:],
                                           op0=OP.mult, op1=OP.add)
            lea_ps = pps.tile([1, 1], F32, tag="pps")
            nc.tensor.matmul(lea_ps[:], li_t[:], ones[0:NL, 0:1],
                             start=True, stop=True)
            nc.scalar.copy(asm[0:1, 45:46], lea_ps[:])      # loss_ea partial

            # ---- local 8-way shard sum, then partial Gram dots + norms ----
            XT8 = wk.tile([128, 8 * 2 * 64], BF16, tag="XT8")
            nc.scalar.dma_start(
                XT8[:].rearrange("p (s b c) -> p s b c", s=8, b=2),
                bounce1_out[:].rearrange("(s b p) c -> p s b c", p=128, b=2))
            XT = wk.tile([128, 2 * 64], F32, tag="XT")
            XT3 = XT[:].rearrange("p (b c) -> p b c", b=2)
            nc.vector.tensor_reduce(
                XT3[:, :, :],
                XT8[:].rearrange("p (s b c) -> p b c s", s=8, b=2),
                axis=AX.X, op=OP.add)
            sq = wk.tile([128, 2 * 64], F32, tag="sq")
            nc.vector.tensor_tensor(sq[:], XT[:], XT[:], op=OP.mult)
            sq3 = sq[:].rearrange("p (b c) -> p b c", b=2)
            # asm cols: 0:21 ptmT[j,i], 21:42 btmT[j,i], 42/43/44 norm^2
            # partials (s/t/bw), 45 loss_ea partial
            dots = (
                (slice(0, 21), (lambda b: XT3[:, b, 0:21]),
                 (lambda b: XT3[:, b, 21:42]), C),
                (slice(21, 42), (lambda b: XT3[:, b, 42:63]),
                 (lambda b: XT3[:, b, 21:42]), C),
                (slice(42, 43), (lambda b: sq3[:, b, 0:21]),
                 (lambda b: ones[:, 0:1]), 1),
                (slice(43, 44), (lambda b: sq3[:, b, 21:42]),
                 (lambda b: ones[:, 0:1]), 1),
                (slice(44, 45), (lambda b: sq3[:, b, 42:63]),
                 (lambda b: ones[:, 0:1]), 1),
            )
            for cols, lf, rf, nn_ in dots:
                g_ps = pps.tile([C, nn_], F32, tag="pps")
                for b in range(2):
                    nc.tensor.matmul(g_ps[:], lf(b), rf(b),
                                     start=(b == 0), stop=(b == 1))
                nc.scalar.copy(asm[0:C, cols], g_ps[:])
            nc.scalar.dma_start(bounce2[:, :], asm[:])
            if use_cc:
                nc.gpsimd.collective_compute(
                    "AllGather", OP.bypass,
                    replica_groups=[list(range(NCORES))],
                    ins=[bounce2[:].opt()], outs=[bounce2_out[:].opt()])
            else:
                for s_ in range(8):
                    nc.sync.dma_start(bounce2_out[64 * s_:64 * (s_ + 1), :],
                                      bounce2[:])
            ao8 = wk.tile([64, 8 * 64], F32, tag="ao8")
            nc.scalar.dma_start(
                ao8[:].rearrange("q (s c) -> q s c", s=8),
                bounce2_out[:].rearrange("(s q) c -> q s c", q=64))
            ao = wk.tile([64, 64], F32, tag="ao")
            nc.vector.tensor_reduce(
                ao[:], ao8[:].rearrange("q (s c) -> q c s", s=8),
                axis=AX.X, op=OP.add)

            # ---------------- final combine (tiny, replicated) --------------
            # nrm3 [21, 3] = sqrt of (|s|^2, |t|^2, |bw|^2) per class
            nrm3 = wk.tile([C, 3], F32, tag="nrm3")
            nc.scalar.activation(nrm3[:], ao[0:C, 42:45], ACT.Sqrt)
            # rn_s [21,1] = 1/max(|s_j|, eps)
            nms = wk.tile([C, 1], F32, tag="nms")
            nc.vector.tensor_scalar(out=nms[:], in0=nrm3[:, 0:1],
                                    scalar1=float(EPS), scalar2=None,
                                    op0=OP.max)
            rn_s = wk.tile([C, 1], F32, tag="rns")
            nc.vector.reciprocal(rn_s[:], nms[:])
            # rn_t [21,1] = 1/max(|t_i|, eps)
            nmt = wk.tile([C, 1], F32, tag="nmt")
            nc.vector.tensor_scalar(out=nmt[:], in0=nrm3[:, 1:2],
                                    scalar1=float(EPS), scalar2=None,
                                    op0=OP.max)
            rn_t = wk.tile([C, 1], F32, tag="rnt")
            nc.vector.reciprocal(rn_t[:], nmt[:])
            # sc_bw [21,1] = crec/max(|bw_sums_j|*crec, eps)
            nb = wk.tile([C, 1], F32, tag="nb")
            nc.vector.tensor_scalar(out=nb[:], in0=nrm3[:, 2:3],
                                    scalar1=crec[:], scalar2=float(EPS),
                                    op0=OP.mult, op1=OP.max)
            rnb = wk.tile([C, 1], F32, tag="rnb")
            nc.vector.reciprocal(rnb[:], nb[:])
            sc_bw = wk.tile([C, 1], F32, tag="scbw")
            nc.vector.tensor_tensor(sc_bw[:], rnb[:], crec[:], op=OP.mult)

            d1 = wk.tile([C, C], F32, tag="d1")
            nc.vector.tensor_scalar(out=d1[:], in0=ao[0:C, 0:C],
                                    scalar1=rn_s[:], scalar2=None, op0=OP.mult)
            dsbT = wk.tile([C, C], F32, tag="dsbT")
            nc.vector.tensor_scalar(out=dsbT[:], in0=ao[0:C, 21:42],
                                    scalar1=sc_bw[:], scalar2=None,
                                    op0=OP.mult)
            nc.vector.tensor_tensor(dsbT[:], dsbT[:], d1[:], op=OP.subtract)
            # |dsbT| with source-class row 0 zeroed (drops ptm/btm col 0)
            absT = wk.tile([C, C], F32, tag="absT")
            nc.vector.scalar_tensor_tensor(out=absT[:], in0=dsbT[:],
                                           scalar=-1.0, in1=dsbT[:],
                                           op0=OP.mult, op1=OP.max)
            nc.vector.tensor_scalar(out=absT[:], in0=absT[:],
                                    scalar1=rmask0[:], scalar2=None,
                                    op0=OP.mult)
            # col-sums over j, then weight by (exist-mask with row0=0) * rn_t
            ti_ps = pps.tile([C, 1], F32, tag="pps")
            nc.tensor.matmul(ti_ps[:], absT[:], ones[0:C, 0:1],
                             start=True, stop=True)
            tot2 = wk.tile([C, 1], F32, tag="tot2")
            nc.vector.scalar_tensor_tensor(out=tot2[:], in0=ti_ps[:],
                                           scalar=cem0[:], in1=rn_t[:],
                                           op0=OP.mult, op1=OP.mult)
            tot_ps = pps.tile([1, 1], F32, tag="pps")
            nc.tensor.matmul(tot_ps[:], tot2[:], ones[0:C, 0:1],
                             start=True, stop=True)
            nm_ps = pps.tile([1, 1], F32, tag="pps")
            nc.tensor.matmul(nm_ps[:], cem0[:], ones[0:C, 0:1],
                             start=True, stop=True)
            nm_sb = wk.tile([1, 1], F32, tag="nmsb")
            nc.scalar.copy(nm_sb[:], nm_ps[:])
            rnm = wk.tile([1, 1], F32, tag="rnm")
            nc.vector.reciprocal(rnm[:], nm_sb[:])

            res = wk.tile([1, 2], F32, tag="res")
            nc.vector.tensor_scalar(out=res[:, 0:1], in0=tot_ps[:],
                                    scalar1=rnm[:],
                                    scalar2=MGRM_W / (C - 1.0),
                                    op0=OP.mult, op1=OP.mult)
            nc.vector.tensor_scalar(out=res[:, 1:2], in0=ao[0:1, 45:46],
                                    scalar1=1.0 / N, scalar2=None,
                                    op0=OP.mult)
            nc.scalar.dma_start(out_dr[:, :], res[:])

    nc.compile()
    return nc


_NC_CACHE = {}
_last_in_maps = None


def _prep_in_maps(inputs):
    feats = np.asarray(inputs["ins_features"], np.float32)
    logits = np.ascontiguousarray(inputs["class_logits"], dtype=np.float32)
    labels = np.ascontiguousarray(inputs["labels"], dtype=np.int32)
    dom = np.ascontiguousarray(inputs["domain_labels"], dtype=np.int32)
    W1 = np.asarray(inputs["W1"], np.float32)
    W1p = np.ascontiguousarray(
        np.concatenate([W1[:D][PERM], W1[D:]], axis=0)).astype(BF)
    W2 = np.ascontiguousarray(inputs["W2"], np.float32).astype(BF)
    W3 = np.ascontiguousarray(inputs["W3"], np.float32).astype(BF)
    Wd = np.asarray(inputs["Wd"], np.float32).reshape(1, H1)
    Wdb = np.ascontiguousarray(np.broadcast_to(Wd, (NL, H1))).astype(BF)
    b1 = np.ascontiguousarray(inputs["b1"], np.float32).reshape(1, H1).astype(BF)
    b2 = np.ascontiguousarray(inputs["b2"], np.float32).reshape(1, H1).astype(BF)
    b3 = np.ascontiguousarray(inputs["b3"], np.float32).reshape(1, H1).astype(BF)
    bd = np.full((NL, 1), float(np.asarray(inputs["bd"]).reshape(-1)[0]),
                 np.float32)

    # pad the 7x7 window to 50 taps (tap 49 = 0) for even-stride PE pooling
    feats_p = np.zeros((N, D, 50), BF)
    feats_p[:, :, :49] = feats.reshape(N, D, 49)

    in_maps = []
    for k in range(NCORES):
        r0 = NL * k
        sel_s = np.zeros((NS, NL), np.float32)
        sel_t = np.zeros((NS, NL), np.float32)
        if r0 + NL <= NS:
            sel_s[np.arange(r0, r0 + NL), np.arange(NL)] = 1.0
        else:
            sel_t[np.arange(r0 - NS, r0 - NS + NL), np.arange(NL)] = 1.0
        in_maps.append({
            "feat_shard": np.ascontiguousarray(feats_p[r0:r0 + NL]),
            "logits_full": logits,
            "logitsT_loc": np.ascontiguousarray(
                logits[r0:r0 + NL].T).astype(BF),
            "labels_in": labels,
            "dom_shard": np.ascontiguousarray(dom[r0:r0 + NL].reshape(NL, 1)),
            "sel_src": sel_s,
            "sel_tgt": sel_t,
            "W1p": W1p, "W2in": W2, "W3in": W3, "Wdb": Wdb,
            "b1in": b1, "b2in": b2, "b3in": b3, "bdin": bd,
        })
    return in_maps


def kernel(**inputs) -> np.ndarray:
    if "nc" not in _NC_CACHE:
        _NC_CACHE["nc"] = _build()
    nc = _NC_CACHE["nc"]
    in_maps = _prep_in_maps(inputs)
    global _last_in_maps
    _last_in_maps = in_maps
    res = bass_utils.run_bass_kernel_spmd(nc, in_maps,
                                          core_ids=list(range(NCORES)))
    return res.results[0]["out_loss"].reshape(2).astype(np.float32)


# revision 49
# speedup vs baseline: 1.1274x; 1.1274x over previous
"""CategoryAwareDAHEAD Trainium2 kernel (8-core SPMD, data-parallel over ROIs).

v2 strategy (roofline: ~21.4MB HBM/core => ~60us stream + short tail)
---------------------------------------------------------------------
* ins_features cast to bf16 HOST-side and sharded 64 rows/core; streamed in
  4 DMAs of 3.2MB on the sync (SP) HWDGE ring, pooled 7x7 by DVE reduces
  into opT [128, 16*64] f32 (feature f = 128*g + p, a fixed permutation of
  original d; W1's rows are permuted identically host-side).
* W1/W2/W3/Wd cast to bf16 host-side, queued on the SAME sync ring right
  AFTER the features (FIFO per HWDGE ring => features stream first, weights
  arrive exactly when the EA matmuls need them). All weight tiles persist
  in SBUF. EA chain (3x matmul+LN+transpose) runs bf16 with f32 PSUM.
* The sequential threshold-gated EMA scan is restructured as in v1:
  th <- max(th, (th+m)/2) run per-class on [21, 2S] tables built with
  onehot/prefix-sum matmuls; per-sample weights w = 1/(j*...*k) via
  exp(-suffix_sum(A*ln(J))) matmuls. All from f32 logits (exact gating).
* Cross-core reduction is SPLIT:
  - MGRM partial prototypes are produced TRANSPOSED ([2048 d-rows, 64]:
    cols = [s_protos | t_protos | bwl_sums | pad]) and ReduceScatter'd, so
    each core owns a fully-reduced D-slice [256, 64]. Each core computes
    partial Gram dots (ptm/btm [21,21]) + norm partials on its slice.
  - One tiny AllReduce [64, 64] then carries dot partials + norm partials
    + the loss_ea partial. The EA branch overlaps the ReduceScatter.
  Cosines are recovered as dot * rn_row * rn_col (scale-invariant, the
  1/49 pooling scale and 1/count bwl scale fold into the rn factors).

KLEVEL env: full (default) | nocc (replace collectives with local copies)
"""

import os
import sys

for _p in ("/opt/trn_rl_repo", "/root/.axon_site/_ro/trn_rl_repo"):
    if _p not in sys.path:
        sys.path.insert(0, _p)

import numpy as np
import ml_dtypes

import concourse.bacc as bacc
import concourse.mybir as mybir
import concourse.tile as tile
from concourse import bass_utils
from concourse.masks import make_identity, make_upper_triangular, make_lower_triangular

F32 = mybir.dt.float32
BF16 = mybir.dt.bfloat16
I32 = mybir.dt.int32
AX = mybir.AxisListType
OP = mybir.AluOpType
ACT = mybir.ActivationFunctionType
BF = ml_dtypes.bfloat16

NCORES = 8
N, NS, C, D = 512, 256, 21, 2048
NL = N // NCORES          # 64 rows per core
H1 = 1024
DIN = D + C               # 2069
S = 32                    # per-class sequence table length (max count is 18)
GN = 8                    # samples per pooling DMA (8 DMAs x 1.6MB bf16)
THR0, MOM, EPS, LN_EPS, MGRM_W = 0.1, 0.5, 1e-8, 1e-5, 1.0
NT = D // 128             # 16 feature blocks

# feature permutation: new index f holds original d = (f % 128) * 16 + f // 128
PERM = (np.arange(D) % 128) * 16 + np.arange(D) // 128


def _build():
    lv = os.environ.get("KLEVEL", "full")
    use_cc = lv != "nocc"

    nc = bacc.Bacc("TRN2", target_bir_lowering=False, debug=False,
                   num_devices=NCORES)

    # ---------------- DRAM I/O ----------------
    # features host-padded 7x7=49 -> 50 per (n, d) so every PE pooling
    # operand has even element offsets/strides (2-byte dtype)
    feat_dr = nc.dram_tensor("feat_shard", [NL, D, 50], BF16, kind="ExternalInput")
    lg_dr = nc.dram_tensor("logits_full", [N, C], F32, kind="ExternalInput")
    lgT_dr = nc.dram_tensor("logitsT_loc", [C, NL], BF16, kind="ExternalInput")
    lab_dr = nc.dram_tensor("labels_in", [NS], I32, kind="ExternalInput")
    dom_dr = nc.dram_tensor("dom_shard", [NL, 1], I32, kind="ExternalInput")
    sels_dr = nc.dram_tensor("sel_src", [NS, NL], F32, kind="ExternalInput")
    selt_dr = nc.dram_tensor("sel_tgt", [NS, NL], F32, kind="ExternalInput")
    w1_dr = nc.dram_tensor("W1p", [DIN, H1], BF16, kind="ExternalInput")
    w2_dr = nc.dram_tensor("W2in", [H1, H1], BF16, kind="ExternalInput")
    w3_dr = nc.dram_tensor("W3in", [H1, H1], BF16, kind="ExternalInput")
    wdb_dr = nc.dram_tensor("Wdb", [NL, H1], BF16, kind="ExternalInput")
    b1_dr = nc.dram_tensor("b1in", [1, H1], BF16, kind="ExternalInput")
    b2_dr = nc.dram_tensor("b2in", [1, H1], BF16, kind="ExternalInput")
    b3_dr = nc.dram_tensor("b3in", [1, H1], BF16, kind="ExternalInput")
    bd_dr = nc.dram_tensor("bdin", [NL, 1], F32, kind="ExternalInput")
    out_dr = nc.dram_tensor("out_loss", [1, 2], F32, kind="ExternalOutput")

    with tile.TileContext(nc) as tc:
        with (
            tc.tile_pool(name="consts", bufs=1) as cst,
            tc.tile_pool(name="insb", bufs=1) as insb,
            tc.tile_pool(name="featp", bufs=4) as featp,
            tc.tile_pool(name="persist", bufs=1) as per,
            tc.tile_pool(name="wpool", bufs=1) as wp,
            tc.tile_pool(name="work", bufs=1) as wk,
            tc.tile_pool(name="pps", bufs=2, space="PSUM") as pps,
            tc.tile_pool(name="ppool", bufs=4, space="PSUM") as ppool,
            tc.tile_pool(name="pph", bufs=1, space="PSUM") as pph,
            tc.tile_pool(name="dram", bufs=1, space="DRAM") as drp,
        ):
            # ---------------- constants ----------------
            id128 = cst.tile([128, 128], F32, tag="id128")
            make_identity(nc, id128[:])
            id128b = cst.tile([128, 128], BF16, tag="id128b")
            make_identity(nc, id128b[:])
            ut128 = cst.tile([128, 128], F32, tag="ut128")
            make_upper_triangular(nc, ut128[:], val=1.0, diag=True)
            lt128 = cst.tile([128, 128], F32, tag="lt128")
            make_lower_triangular(nc, lt128[:], val=1.0, diag=True)
            ones = cst.tile([128, 128], F32, tag="ones")
            nc.gpsimd.memset(ones[:], 1.0)
            iotaS = cst.tile([128, S], F32, tag="iotaS")   # 1..S per partition
            nc.gpsimd.iota(iotaS[:], [[1, S]], base=1, channel_multiplier=0,
                           allow_small_or_imprecise_dtypes=True)
            iota21 = cst.tile([128, C], F32, tag="iota21")  # 0..20
            nc.gpsimd.iota(iota21[:], [[1, C]], base=0, channel_multiplier=0,
                           allow_small_or_imprecise_dtypes=True)
            epsln = cst.tile([128, 1], F32, tag="epsln")
            nc.gpsimd.memset(epsln[:], LN_EPS)
            onesb = cst.tile([1, 128], BF16, tag="onesb")
            nc.gpsimd.memset(onesb[:], 1.0)

            # ------------- small input DMAs (ACT HWDGE ring) -------------
            lg_sb = insb.tile([128, 4 * C], F32, tag="lg")       # [128, 84]
            nc.scalar.dma_start(
                lg_sb[:].rearrange("p (c l) -> p c l", c=4),
                lg_dr.ap().rearrange("(c p) l -> p c l", p=128))
            lgT_sb = insb.tile([C, NL], BF16, tag="lgT")
            nc.scalar.dma_start(lgT_sb[:], lgT_dr[:, :])
            lab_i = insb.tile([128, 2], I32, tag="labi")
            nc.scalar.dma_start(lab_i[:], lab_dr.ap().rearrange(
                "(c p) -> p c", p=128))
            dom_i = insb.tile([NL, 1], I32, tag="domi")
            nc.scalar.dma_start(dom_i[:], dom_dr[:, :])
            sels_sb = insb.tile([128, 2 * NL], F32, tag="sels")
            nc.scalar.dma_start(
                sels_sb[:].rearrange("p (c n) -> p c n", c=2),
                sels_dr.ap().rearrange("(c p) n -> p c n", p=128))
            selt_sb = insb.tile([128, 2 * NL], F32, tag="selt")
            nc.scalar.dma_start(
                selt_sb[:].rearrange("p (c n) -> p c n", c=2),
                selt_dr.ap().rearrange("(c p) n -> p c n", p=128))
            b1_sb = insb.tile([1, H1], BF16, tag="b1")
            nc.scalar.dma_start(b1_sb[:], b1_dr[:, :])
            b2_sb = insb.tile([1, H1], BF16, tag="b2")
            nc.scalar.dma_start(b2_sb[:], b2_dr[:, :])
            b3_sb = insb.tile([1, H1], BF16, tag="b3")
            nc.scalar.dma_start(b3_sb[:], b3_dr[:, :])
            bd_sb = insb.tile([NL, 1], F32, tag="bd")
            nc.scalar.dma_start(bd_sb[:], bd_dr[:, :])
            wdb_sb = insb.tile([NL, H1], BF16, tag="wdb")
            nc.scalar.dma_start(wdb_sb[:], wdb_dr[:, :])

            lab_f = insb.tile([128, 2], F32, tag="labf")
            nc.vector.tensor_copy(lab_f[:], lab_i[:])
            dom_f = insb.tile([NL, 1], F32, tag="domf")
            nc.vector.tensor_copy(dom_f[:], dom_i[:])

            # -------- feature stream + pooling (SP HWDGE ring, FIFO) --------
            # feat_dr [NL, 2048, 7, 7] bf16 viewed as [128(p), NL, 784] where
            # partition p holds original d in [16p, 16p+16), 784 = 16*49.
            # Pooling on the PE: 25 accumulating matmuls with a stationary
            # identity sum the (padded) 7x7 window two taps at a time at
            # 128 elem/cycle, freeing the DVE for the threshold-chain work
            # that runs concurrently. Even offsets/strides throughout.
            feat_ap = feat_dr.ap().rearrange("n (p g) w -> p n (g w)", p=128)
            opT = per.tile([128, 16 * NL], F32, tag="opT")   # RAW sums (x49)
            opT3 = opT[:].rearrange("p (g n) -> p g n", g=16)
            fts = []
            for n0 in range(0, NL, GN):
                ft = featp.tile([128, GN * 800], BF16, tag="ft")
                nc.sync.dma_start(ft[:], feat_ap[:, n0:n0 + GN, :])
                fts.append(ft)
            # (pooling matmuls are emitted AFTER the threshold-chain section
            # below: the PE runs its queue strictly in order, and the chain's
            # small matmuls must not sit behind 100 pooling matmuls)

            # -------- weight prefetch (queued behind features, same ring) ----
            w1_sb = []
            for kc in range(17):
                kp = 128 if kc < 16 else C
                wt = wp.tile([128, H1], BF16, tag=f"w1_{kc}")
                nc.sync.dma_start(wt[0:kp, :], w1_dr[128 * kc:128 * kc + kp, :])
                w1_sb.append(wt)
            w2_sb, w3_sb = [], []
            for w_dr, lst, nm in ((w2_dr, w2_sb, "w2"), (w3_dr, w3_sb, "w3")):
                for kc in range(8):
                    wt = wp.tile([128, H1], BF16, tag=f"{nm}_{kc}")
                    nc.sync.dma_start(wt[:], w_dr[128 * kc:128 * (kc + 1), :])
                    lst.append(wt)

            # ---------------- per-chunk softmax stats (f32, exact) ----------
            E_ch, OHP_ch, mlOHP_ch = [], [], []
            for ch in range(4):
                lg_c = lg_sb[:, C * ch:C * (ch + 1)]
                mx = wk.tile([128, 1], F32, tag=f"mx{ch}")
                nc.vector.tensor_reduce(mx[:], lg_c, axis=AX.X, op=OP.max)
                E = wk.tile([128, C], F32, tag=f"E{ch}")
                nc.vector.tensor_scalar(out=E[:], in0=lg_c, scalar1=mx[:],
                                        scalar2=None, op0=OP.is_equal)
                negmx = wk.tile([128, 1], F32, tag=f"nmx{ch}")
                nc.vector.tensor_scalar_mul(negmx[:], mx[:], -1.0)
                scr = wk.tile([128, C], F32, tag=f"scr{ch}")
                den = wk.tile([128, 1], F32, tag=f"den{ch}")
                nc.scalar.activation(scr[:], lg_c, ACT.Exp, bias=negmx[:],
                                     scale=1.0, accum_out=den[:])
                ml = wk.tile([128, 1], F32, tag=f"ml{ch}")
                nc.vector.reciprocal(ml[:], den[:])
                E_ch.append(E)

                dom = ch // 2   # 0 = src, 1 = tgt
                P_ps = pps.tile([128, C], F32, tag="pps")
                if ch % 2 == 0:
                    nc.tensor.matmul(P_ps[:], ut128[:], E[:], start=True,
                                     stop=True)
                else:
                    nc.tensor.matmul(P_ps[:], ones[:], E_ch[2 * dom][:],
                                     start=True, stop=False)
                    nc.tensor.matmul(P_ps[:], ut128[:], E[:], start=False,
                                     stop=True)
                pos = wk.tile([128, 1], F32, tag=f"pos{ch}")
                posscr = wk.tile([128, C], F32, tag=f"poss{ch}")
                nc.vector.tensor_tensor(posscr[:], P_ps[:], E[:], op=OP.mult)
                nc.vector.tensor_reduce(pos[:], posscr[:], axis=AX.X,
                                        op=OP.add)
                OHP = wk.tile([128, S], F32, tag=f"OHP{ch}")
                nc.vector.tensor_scalar(out=OHP[:], in0=iotaS[:],
                                        scalar1=pos[:], scalar2=None,
                                        op0=OP.is_equal)
                mlOHP = wk.tile([128, S], F32, tag=f"mlO{ch}")
                nc.vector.tensor_scalar(out=mlOHP[:], in0=iotaS[:],
                                        scalar1=pos[:], scalar2=ml[:],
                                        op0=OP.is_equal, op1=OP.mult)
                OHP_ch.append(OHP)
                mlOHP_ch.append(mlOHP)

            # tables T[21, 2S]: cols 0..S-1 src, S..2S-1 tgt (dom-major)
            T_all = wk.tile([C, 2 * S], F32, tag="Tall")
            for dom in range(2):
                T_ps = pps.tile([C, S], F32, tag="pps")
                nc.tensor.matmul(T_ps[:], E_ch[2 * dom][:],
                                 mlOHP_ch[2 * dom][:], start=True, stop=False)
                nc.tensor.matmul(T_ps[:], E_ch[2 * dom + 1][:],
                                 mlOHP_ch[2 * dom + 1][:], start=False,
                                 stop=True)
                nc.scalar.copy(T_all[:, S * dom:S * (dom + 1)], T_ps[:])

            # E-transposes for the EW gathers (depend only on E_ch: emit
            # early so they do not serialize the later EW section)
            ET_ch = []
            for ch in range(4):
                ET = wk.tile([C, 128], F32, tag=f"ET{ch}")
                et_ps = pps.tile([C, 128], F32, tag="pps")
                nc.tensor.transpose(et_ps[:], E_ch[ch][:], id128[:])
                nc.scalar.copy(ET[:], et_ps[:])
                ET_ch.append(ET)

            # pooling matmuls, interleaved into the chain's DVE-bound gaps
            # (the PE runs strictly in order; 25 accumulating matmuls per
            # group sum the padded 7x7 window two taps at a time)
            pool_pss = []

            def emit_pool_group(gi):
                ftj = fts[gi][:].rearrange("p (x j k) -> p j x k", j=25, k=2)
                pool_ps = ppool.tile([128, GN * 16 * 2], F32, tag="pool")
                pool2 = pool_ps[:].rearrange("p (x k) -> p x k", k=2)
                for j in range(25):
                    nc.tensor.matmul(pool2, id128b[:], ftj[:, j, :, :],
                                     start=(j == 0), stop=(j == 24))
                pool_pss.append(pool_ps)

            def emit_recombine(gi):
                rn0 = gi * GN
                rp2 = pool_pss[gi][:].rearrange("p (x k) -> p x k", k=2)
                nc.vector.tensor_reduce(
                    opT3[:, :, rn0:rn0 + GN].rearrange("p g n -> p n g"),
                    rp2, axis=AX.X, op=OP.add)

            for _g in (0, 1, 2):
                emit_pool_group(_g)

            # ------------- sequential threshold chain -------------
            T3 = T_all[:].rearrange("p (d s) -> p d s", d=2)
            th = wk.tile([C, 2], F32, tag="th")
            nc.gpsimd.memset(th[:], THR0)
            A = wk.tile([C, 2 * S], F32, tag="A")
            A3 = A[:].rearrange("p (d s) -> p d s", d=2)
            tmp = wk.tile([C, 2], F32, tag="chtmp")
            for s in range(S):
                m = T3[:, :, s]
                nc.vector.tensor_tensor(A3[:, :, s], m, th[:], op=OP.is_ge)
                nc.vector.tensor_tensor(tmp[:], m, th[:], op=OP.add)
                nc.vector.scalar_tensor_tensor(
                    out=th[:], in0=tmp[:], scalar=0.5, in1=th[:],
                    op0=OP.mult, op1=OP.max)

            for _g in (0, 1, 2):
                emit_recombine(_g)
            for _g in (3, 4):
                emit_pool_group(_g)

            # per-domain accept-index math -> wtab [21, 2S]
            wtab = wk.tile([C, 2 * S], F32, tag="wtab")
            for dom in range(2):
                A_dom = A[:, S * dom:S * (dom + 1)]
                A_T = wk.tile([S, C], F32, tag=f"AT{dom}")
                at_ps = pps.tile([S, C], F32, tag="pps")
                nc.tensor.transpose(at_ps[:], A_dom, id128[0:C, 0:C])
                nc.scalar.copy(A_T[:], at_ps[:])
                J_ps = pps.tile([C, S], F32, tag="pps")
                nc.tensor.matmul(J_ps[:], A_T[:], ut128[0:S, 0:S],
                                 start=True, stop=True)
                jc = wk.tile([C, S], F32, tag=f"jc{dom}")
                nc.vector.tensor_scalar(out=jc[:], in0=J_ps[:], scalar1=1.0,
                                        scalar2=None, op0=OP.max)
                lnJ = wk.tile([C, S], F32, tag=f"lnJ{dom}")
                nc.scalar.activation(lnJ[:], jc[:], ACT.Ln)
                nc.vector.tensor_tensor(lnJ[:], lnJ[:], A_dom, op=OP.mult)
                lnJ_T = wk.tile([S, C], F32, tag=f"lnJT{dom}")
                lt_ps = pps.tile([S, C], F32, tag="pps")
                nc.tensor.transpose(lt_ps[:], lnJ[:], id128[0:C, 0:C])
                nc.scalar.copy(lnJ_T[:], lt_ps[:])
                SS_ps = pps.tile([C, S], F32, tag="pps")
                nc.tensor.matmul(SS_ps[:], lnJ_T[:], lt128[0:S, 0:S],
                                 start=True, stop=True)
                wt_dom = wtab[:, S * dom:S * (dom + 1)]
                nc.scalar.activation(wt_dom, SS_ps[:], ACT.Exp, scale=-1.0)
                nc.vector.tensor_tensor(wt_dom, wt_dom, A_dom, op=OP.mult)

            for _g in (5, 6, 7):
                emit_pool_group(_g)

            # remaining recombines (emitted late so the DVE queue's chain
            # work is not stalled)
            for _g in (3, 4, 5, 6, 7):
                emit_recombine(_g)

            # -------- opT consumers: bf16 copy (EA) + feat64 (protos) -------
            opTb = per.tile([128, 16 * NL], BF16, tag="opTb")  # scaled 1/49
            nc.vector.tensor_scalar_mul(opTb[:], opT[:], 1.0 / 49.0)

            # feat64 [64, 2048] sample-major (bf16 PE transpose of opTb blocks;
            # the uniform 1/49 scale cancels in every cosine downstream)
            feat64 = per.tile([NL, D], BF16, tag="feat64")
            for half in range(2):
                tp = pps.tile([NL, 1024], BF16, tag="pps")
                for g in range(8):
                    gg = half * 8 + g
                    nc.tensor.transpose(tp[:, 128 * g:128 * (g + 1)],
                                        opTb[:, NL * gg:NL * (gg + 1)],
                                        id128b[:])
                nc.scalar.copy(feat64[:, 1024 * half:1024 * (half + 1)], tp[:])

            # ---------- per-sample weights, local gathers ----------
            # EWl_all [64, 64]: cols 0:21 src-proto w, 21:42 tgt-proto w,
            # 42:63 label onehot, 63 pad(0)
            EWl_all = wk.tile([NL, 64], BF16, tag="EWall")
            nc.gpsimd.memset(EWl_all[:], 0.0)
            for dom in range(2):
                sel = sels_sb if dom == 0 else selt_sb
                ewl_ps = pps.tile([NL, C], F32, tag="pps")
                for cc in range(2):
                    ch = 2 * dom + cc
                    G_ps = pps.tile([128, S], F32, tag="pps")
                    nc.tensor.matmul(G_ps[:], ET_ch[ch][:],
                                     wtab[:, S * dom:S * (dom + 1)],
                                     start=True, stop=True)
                    ws = wk.tile([128, 1], F32, tag=f"ws{ch}")
                    wscr = wk.tile([128, S], F32, tag=f"wscr{ch}")
                    nc.vector.tensor_tensor(wscr[:], G_ps[:], OHP_ch[ch][:],
                                            op=OP.mult)
                    nc.vector.tensor_reduce(ws[:], wscr[:], axis=AX.X,
                                            op=OP.add)
                    EW = wk.tile([128, C], F32, tag=f"EW{ch}")
                    nc.vector.tensor_scalar(out=EW[:], in0=E_ch[ch][:],
                                            scalar1=ws[:], scalar2=None,
                                            op0=OP.mult)
                    nc.tensor.matmul(ewl_ps[:], sel[:, NL * cc:NL * (cc + 1)],
                                     EW[:], start=(cc == 0), stop=(cc == 1))
                nc.scalar.copy(EWl_all[:, 21 * dom:21 * (dom + 1)], ewl_ps[:])

            # labels onehot + counts + local label gather
            elab_ps = pps.tile([NL, C], F32, tag="pps")
            cnt_ps = pps.tile([C, 1], F32, tag="pps")
            for cc in range(2):
                Elab = wk.tile([128, C], F32, tag=f"Elab{cc}")
                nc.vector.tensor_scalar(out=Elab[:], in0=iota21[:],
                                        scalar1=lab_f[:, cc:cc + 1],
                                        scalar2=None, op0=OP.is_equal)
                nc.tensor.matmul(cnt_ps[:], Elab[:], ones[:, 0:1],
                                 start=(cc == 0), stop=(cc == 1))
                nc.tensor.matmul(elab_ps[:], sels_sb[:, NL * cc:NL * (cc + 1)],
                                 Elab[:], start=(cc == 0), stop=(cc == 1))
            nc.scalar.copy(EWl_all[:, 42:63], elab_ps[:])
            counts = wk.tile([C, 1], F32, tag="counts")
            nc.scalar.copy(counts[:], cnt_ps[:])

            # count-derived scalars for the final combine (hoisted: DVE is
            # free here, and gpsimd memsets must precede the collective
            # waits that block the gpsimd queue)
            cexist = wk.tile([C, 1], F32, tag="cex")
            nc.vector.tensor_scalar(out=cexist[:], in0=counts[:], scalar1=0.0,
                                    scalar2=None, op0=OP.is_gt)
            cem0 = wk.tile([C, 1], F32, tag="cem0")
            nc.vector.tensor_copy(cem0[:], cexist[:])
            nc.gpsimd.memset(cem0[0:1, :], 0.0)
            rmask0 = wk.tile([C, 1], F32, tag="rmask0")
            nc.gpsimd.memset(rmask0[:], 1.0)
            nc.gpsimd.memset(rmask0[0:1, :], 0.0)
            cmax = wk.tile([C, 1], F32, tag="cmax")
            nc.vector.tensor_scalar(out=cmax[:], in0=counts[:], scalar1=1.0,
                                    scalar2=None, op0=OP.max)
            crec = wk.tile([C, 1], F32, tag="crec")
            nc.vector.reciprocal(crec[:], cmax[:])

            # ---- transposed partial protos prT [2048, 64] -> AllToAll ----
            # (A2A = pure copies on the fast mesh path; the 8-way shard sum
            # happens locally on DVE. Equivalent to ReduceScatter but ~4x
            # cheaper at this size on the 8-rank ring.)
            bounce1 = drp.tile([D, 64], BF16, tag="bounce1")
            bounce1_out = drp.tile([D, 64], BF16, tag="bounce1o")
            prT_ps = pph.tile([128, 16 * 64], F32, tag="pph")
            for t in range(NT):
                nc.tensor.matmul(prT_ps[:, 64 * t:64 * (t + 1)],
                                 feat64[:, 128 * t:128 * (t + 1)],
                                 EWl_all[:], start=True, stop=True)
            # payload tile for the tiny AllReduce (filled piecewise below);
            # memset BEFORE the RS trigger: gpsimd blocks on collective waits
            asm = wk.tile([64, 64], F32, tag="asm")
            nc.gpsimd.memset(asm[:], 0.0)

            prT_sb = per.tile([128, 16 * 64], BF16, tag="prTsb")
            nc.scalar.copy(prT_sb[:], prT_ps[:])
            nc.scalar.dma_start(
                bounce1[:].rearrange("(t p) c -> p t c", p=128),
                prT_sb[:].rearrange("p (t c) -> p t c", t=16))
            if use_cc:
                nc.gpsimd.collective_compute(
                    "AllToAll", OP.bypass,
                    replica_groups=[list(range(NCORES))],
                    ins=[bounce1[:].opt()], outs=[bounce1_out[:].opt()])
            else:
                nc.sync.dma_start(bounce1_out[:], bounce1[:])

            bounce2 = drp.tile([64, 64], F32, tag="bounce2")
            bounce2_out = drp.tile([8 * 64, 64], F32, tag="bounce2o")

            # ---------------- EA branch (bf16, per-core rows) ----------------
            def ln_relu(h_ps, li):
                s1 = wk.tile([NL, 1], F32, tag="s1")
                nc.vector.tensor_reduce(s1[:], h_ps[:], axis=AX.X, op=OP.add)
                s2scr = wk.tile([NL, H1], BF16, tag="s2scr")
                s2 = wk.tile([NL, 1], F32, tag="s2")
                nc.scalar.activation(s2scr[:], h_ps[:], ACT.Square,
                                     accum_out=s2[:])
                mu = wk.tile([NL, 1], F32, tag="mu")
                nc.vector.tensor_scalar_mul(mu[:], s1[:], 1.0 / H1)
                ms2 = wk.tile([NL, 1], F32, tag="ms2")
                nc.vector.tensor_scalar_mul(ms2[:], s2[:], 1.0 / H1)
                nvar = wk.tile([NL, 1], F32, tag="nvar")
                nc.vector.scalar_tensor_tensor(out=nvar[:], in0=mu[:],
                                               scalar=mu[:], in1=ms2[:],
                                               op0=OP.mult, op1=OP.subtract)
                sd = wk.tile([NL, 1], F32, tag="sd")
                nc.scalar.activation(sd[:], nvar[:], ACT.Sqrt, scale=-1.0,
                                     bias=epsln[0:NL, :])
                rstd = wk.tile([NL, 1], F32, tag="rstd")
                nc.vector.reciprocal(rstd[:], sd[:])
                mb = wk.tile([NL, 1], F32, tag="mb")
                nc.vector.tensor_scalar(out=mb[:], in0=mu[:], scalar1=rstd[:],
                                        scalar2=-1.0, op0=OP.mult, op1=OP.mult)
                h = wk.tile([NL, H1], BF16, tag=f"h{li}")
                nc.scalar.activation(h[:], h_ps[:], ACT.Relu, bias=mb[:],
                                     scale=rstd[:])
                return h

            def transpose_h2(h, li):
                hT = wk.tile([128, 8 * NL], BF16, tag=f"hT{li}")
                for half in range(2):
                    ht_ps = pps.tile([128, 4 * NL], BF16, tag="pps")
                    for j in range(4):
                        jj = 4 * half + j
                        nc.tensor.transpose(ht_ps[:, NL * j:NL * (j + 1)],
                                            h[:, 128 * jj:128 * (jj + 1)],
                                            id128b[0:NL, 0:NL])
                    nc.scalar.copy(hT[:, 4 * NL * half:4 * NL * (half + 1)],
                                   ht_ps[:])
                return hT

            # layer 1: lhsT chunks = opTb blocks + logitsT (all bf16)
            h_ps = pph.tile([NL, H1], F32, tag="pph")
            for half in range(2):
                nsl = slice(512 * half, 512 * (half + 1))
                nc.tensor.matmul(h_ps[:, nsl], onesb[:, 0:NL],
                                 b1_sb[:, nsl], start=True, stop=False)
            for kc in range(17):
                kp = 128 if kc < 16 else C
                lhsT = (opTb[:, NL * kc:NL * (kc + 1)] if kc < 16
                        else lgT_sb[:, :])
                for half in range(2):
                    nsl = slice(512 * half, 512 * (half + 1))
                    nc.tensor.matmul(h_ps[:, nsl], lhsT,
                                     w1_sb[kc][0:kp, nsl],
                                     start=False, stop=(kc == 16))
            h = ln_relu(h_ps, 1)

            for li, (wl, b_sb) in enumerate(((w2_sb, b2_sb), (w3_sb, b3_sb))):
                hT = transpose_h2(h, li + 1)
                h_ps = pph.tile([NL, H1], F32, tag="pph")
                for half in range(2):
                    nsl = slice(512 * half, 512 * (half + 1))
                    nc.tensor.matmul(h_ps[:, nsl], onesb[:, 0:NL],
                                     b_sb[:, nsl], start=True, stop=False)
                for kc in range(8):
                    for half in range(2):
                        nsl = slice(512 * half, 512 * (half + 1))
                        nc.tensor.matmul(h_ps[:, nsl],
                                         hT[:, NL * kc:NL * (kc + 1)],
                                         wl[kc][:, nsl],
                                         start=False, stop=(kc == 7))
                h = ln_relu(h_ps, li + 2)

            # domain head on DVE (avoids a third transpose round)
            zscr = wk.tile([NL, H1], F32, tag="zscr")
            nc.vector.tensor_tensor(zscr[:], h[:], wdb_sb[:], op=OP.mult)
            zd = wk.tile([NL, 1], F32, tag="zd")
            nc.vector.tensor_reduce(zd[:], zscr[:], axis=AX.X, op=OP.add)
            z = wk.tile([NL, 1], F32, tag="z")
            nc.scalar.activation(z[:], zd[:], ACT.Sigmoid, bias=bd_sb[:])
            # BCE-with-logits on z: li = z*(1-y) + ln(1 + exp(-z))  (z >= 0)
            enz = wk.tile([NL, 1], F32, tag="enz")
            nc.scalar.activation(enz[:], z[:], ACT.Exp, scale=-1.0)
            sp = wk.tile([NL, 1], F32, tag="sp")
            nc.scalar.activation(sp[:], enz[:], ACT.Ln, bias=ones[0:NL, 0:1])
            omy = wk.tile([NL, 1], F32, tag="omy")
            nc.vector.tensor_scalar(out=omy[:], in0=dom_f[:], scalar1=-1.0,
                                    scalar2=1.0, op0=OP.mult, op1=OP.add)
            li_t = wk.tile([NL, 1], F32, tag="li")
            nc.vector.scalar_tensor_tensor(out=li_t[:], in0=z[:],
                                           scalar=omy[:], in1=sp[:],
                                           op0=OP.mult, op1=OP.add)
            lea_ps = pps.tile([1, 1], F32, tag="pps")
            nc.tensor.matmul(lea_ps[:], li_t[:], ones[0:NL, 0:1],
                             start=True, stop=True)
            nc.scalar.copy(asm[0:1, 45:46], lea_ps[:])      # loss_ea partial

            # ---- local 8-way shard sum, then partial Gram dots + norms ----
            XT8 = wk.tile([128, 8 * 2 * 64], BF16, tag="XT8")
            nc.scalar.dma_start(
                XT8[:].rearrange("p (s b c) -> p s b c", s=8, b=2),
                bounce1_out[:].rearrange("(s b p) c -> p s b c", p=128, b=2))
            XT8v = XT8[:].rearrange("p (s b c) -> p s b c", s=8, b=2)
            XT = wk.tile([128, 2 * 64], F32, tag="XT")
            XT3 = XT[:].rearrange("p (b c) -> p b c", b=2)
            nc.vector.tensor_copy(XT3[:, :, :], XT8v[:, 0, :, :])
            for s_ in range(1, 8):
                nc.vector.tensor_tensor(XT3[:, :, :], XT3[:, :, :],
                                        XT8v[:, s_, :, :], op=OP.add)
            sq = wk.tile([128, 2 * 64], F32, tag="sq")
            nc.vector.tensor_tensor(sq[:], XT[:], XT[:], op=OP.mult)
            sq3 = sq[:].rearrange("p (b c) -> p b c", b=2)
            # asm cols: 0:21 ptmT[j,i], 21:42 btmT[j,i], 42/43/44 norm^2
            # partials (s/t/bw), 45 loss_ea partial
            dots = (
                (slice(0, 21), (lambda b: XT3[:, b, 0:21]),
                 (lambda b: XT3[:, b, 21:42]), C),
                (slice(21, 42), (lambda b: XT3[:, b, 42:63]),
                 (lambda b: XT3[:, b, 21:42]), C),
                (slice(42, 43), (lambda b: sq3[:, b, 0:21]),
                 (lambda b: ones[:, 0:1]), 1),
                (slice(43, 44), (lambda b: sq3[:, b, 21:42]),
                 (lambda b: ones[:, 0:1]), 1),
                (slice(44, 45), (lambda b: sq3[:, b, 42:63]),
                 (lambda b: ones[:, 0:1]), 1),
            )
            for cols, lf, rf, nn_ in dots:
                g_ps = pps.tile([C, nn_], F32, tag="pps")
                for b in range(2):
                    nc.tensor.matmul(g_ps[:], lf(b), rf(b),
                                     start=(b == 0), stop=(b == 1))
                nc.scalar.copy(asm[0:C, cols], g_ps[:])
            nc.scalar.dma_start(bounce2[:, :], asm[:])
            if use_cc:
                nc.gpsimd.collective_compute(
                    "AllGather", OP.bypass,
                    replica_groups=[list(range(NCORES))],
                    ins=[bounce2[:].opt()], outs=[bounce2_out[:].opt()])
            else:
                for s_ in range(8):
                    nc.sync.dma_start(bounce2_out[64 * s_:64 * (s_ + 1), :],
                                      bounce2[:])
            ao8 = wk.tile([64, 8 * 64], F32, tag="ao8")
            nc.scalar.dma_start(
                ao8[:].rearrange("q (s c) -> q s c", s=8),
                bounce2_out[:].rearrange("(s q) c -> q s c", q=64))
            ao8v = ao8[:].rearrange("q (s c) -> q s c", s=8)
            ao = wk.tile([64, 64], F32, tag="ao")
            nc.vector.tensor_copy(ao[:], ao8v[:, 0, :])
            for s_ in range(1, 8):
                nc.vector.tensor_tensor(ao[:], ao[:], ao8v[:, s_, :],
                                        op=OP.add)

            # ---------------- final combine (tiny, replicated) --------------
            # nrm3 [21, 3] = sqrt of (|s|^2, |t|^2, |bw|^2) per class
            nrm3 = wk.tile([C, 3], F32, tag="nrm3")
            nc.scalar.activation(nrm3[:], ao[0:C, 42:45], ACT.Sqrt)
            # rn_s [21,1] = 1/max(|s_j|, eps)
            nms = wk.tile([C, 1], F32, tag="nms")
            nc.vector.tensor_scalar(out=nms[:], in0=nrm3[:, 0:1],
                                    scalar1=float(EPS), scalar2=None,
                                    op0=OP.max)
            rn_s = wk.tile([C, 1], F32, tag="rns")
            nc.vector.reciprocal(rn_s[:], nms[:])
            # rn_t [21,1] = 1/max(|t_i|, eps)
            nmt = wk.tile([C, 1], F32, tag="nmt")
            nc.vector.tensor_scalar(out=nmt[:], in0=nrm3[:, 1:2],
                                    scalar1=float(EPS), scalar2=None,
                                    op0=OP.max)
            rn_t = wk.tile([C, 1], F32, tag="rnt")
            nc.vector.reciprocal(rn_t[:], nmt[:])
            # sc_bw [21,1] = crec/max(|bw_sums_j|*crec, eps)
            nb = wk.tile([C, 1], F32, tag="nb")
            nc.vector.tensor_scalar(out=nb[:], in0=nrm3[:, 2:3],
                                    scalar1=crec[:], scalar2=float(EPS),
                                    op0=OP.mult, op1=OP.max)
            rnb = wk.tile([C, 1], F32, tag="rnb")
            nc.vector.reciprocal(rnb[:], nb[:])
            sc_bw = wk.tile([C, 1], F32, tag="scbw")
            nc.vector.tensor_tensor(sc_bw[:], rnb[:], crec[:], op=OP.mult)

            d1 = wk.tile([C, C], F32, tag="d1")
            nc.vector.tensor_scalar(out=d1[:], in0=ao[0:C, 0:C],
                                    scalar1=rn_s[:], scalar2=None, op0=OP.mult)
            dsbT = wk.tile([C, C], F32, tag="dsbT")
            nc.vector.tensor_scalar(out=dsbT[:], in0=ao[0:C, 21:42],
                                    scalar1=sc_bw[:], scalar2=None,
                                    op0=OP.mult)
            nc.vector.tensor_tensor(dsbT[:], dsbT[:], d1[:], op=OP.subtract)
            # |dsbT| with source-class row 0 zeroed (drops ptm/btm col 0)
            absT = wk.tile([C, C], F32, tag="absT")
            nc.vector.scalar_tensor_tensor(out=absT[:], in0=dsbT[:],
                                           scalar=-1.0, in1=dsbT[:],
                                           op0=OP.mult, op1=OP.max)
            nc.vector.tensor_scalar(out=absT[:], in0=absT[:],
                                    scalar1=rmask0[:], scalar2=None,
                                    op0=OP.mult)
            # col-sums over j, then weight by (exist-mask with row0=0) * rn_t
            ti_ps = pps.tile([C, 1], F32, tag="pps")
            nc.tensor.matmul(ti_ps[:], absT[:], ones[0:C, 0:1],
                             start=True, stop=True)
            tot2 = wk.tile([C, 1], F32, tag="tot2")
            nc.vector.scalar_tensor_tensor(out=tot2[:], in0=ti_ps[:],
                                           scalar=cem0[:], in1=rn_t[:],
                                           op0=OP.mult, op1=OP.mult)
            tot_ps = pps.tile([1, 1], F32, tag="pps")
            nc.tensor.matmul(tot_ps[:], tot2[:], ones[0:C, 0:1],
                             start=True, stop=True)
            nm_ps = pps.tile([1, 1], F32, tag="pps")
            nc.tensor.matmul(nm_ps[:], cem0[:], ones[0:C, 0:1],
                             start=True, stop=True)
            nm_sb = wk.tile([1, 1], F32, tag="nmsb")
            nc.scalar.copy(nm_sb[:], nm_ps[:])
            rnm = wk.tile([1, 1], F32, tag="rnm")
            nc.vector.reciprocal(rnm[:], nm_sb[:])

            res = wk.tile([1, 2], F32, tag="res")
            nc.vector.tensor_scalar(out=res[:, 0:1], in0=tot_ps[:],
                                    scalar1=rnm[:],
                                    scalar2=MGRM_W / (C - 1.0),
                                    op0=OP.mult, op1=OP.mult)
            nc.vector.tensor_scalar(out=res[:, 1:2], in0=ao[0:1, 45:46],
                                    scalar1=1.0 / N, scalar2=None,
                                    op0=OP.mult)
            nc.scalar.dma_start(out_dr[:, :], res[:])

    nc.compile()
    return nc


_NC_CACHE = {}
_last_in_maps = None


def _prep_in_maps(inputs):
    feats = np.asarray(inputs["ins_features"], np.float32)
    logits = np.ascontiguousarray(inputs["class_logits"], dtype=np.float32)
    labels = np.ascontiguousarray(inputs["labels"], dtype=np.int32)
    dom = np.ascontiguousarray(inputs["domain_labels"], dtype=np.int32)
    W1 = np.asarray(inputs["W1"], np.float32)
    W1p = np.ascontiguousarray(
        np.concatenate([W1[:D][PERM], W1[D:]], axis=0)).astype(BF)
    W2 = np.ascontiguousarray(inputs["W2"], np.float32).astype(BF)
    W3 = np.ascontiguousarray(inputs["W3"], np.float32).astype(BF)
    Wd = np.asarray(inputs["Wd"], np.float32).reshape(1, H1)
    Wdb = np.ascontiguousarray(np.broadcast_to(Wd, (NL, H1))).astype(BF)
    b1 = np.ascontiguousarray(inputs["b1"], np.float32).reshape(1, H1).astype(BF)
    b2 = np.ascontiguousarray(inputs["b2"], np.float32).reshape(1, H1).astype(BF)
    b3 = np.ascontiguousarray(inputs["b3"], np.float32).reshape(1, H1).astype(BF)
    bd = np.full((NL, 1), float(np.asarray(inputs["bd"]).reshape(-1)[0]),
                 np.float32)

    # pad the 7x7 window to 50 taps (tap 49 = 0) for even-stride PE pooling
    feats_p = np.zeros((N, D, 50), BF)
    feats_p[:, :, :49] = feats.reshape(N, D, 49)

    in_maps = []
    for k in range(NCORES):
        r0 = NL * k
        sel_s = np.zeros((NS, NL), np.float32)
        sel_t = np.zeros((NS, NL), np.float32)
        if r0 + NL <= NS:
            sel_s[np.arange(r0, r0 + NL), np.arange(NL)] = 1.0
        else:
            sel_t[np.arange(r0 - NS, r0 - NS + NL), np.arange(NL)] = 1.0
        in_maps.append({
            "feat_shard": np.ascontiguousarray(feats_p[r0:r0 + NL]),
            "logits_full": logits,
            "logitsT_loc": np.ascontiguousarray(
                logits[r0:r0 + NL].T).astype(BF),
            "labels_in": labels,
            "dom_shard": np.ascontiguousarray(dom[r0:r0 + NL].reshape(NL, 1)),
            "sel_src": sel_s,
            "sel_tgt": sel_t,
            "W1p": W1p, "W2in": W2, "W3in": W3, "Wdb": Wdb,
            "b1in": b1, "b2in": b2, "b3in": b3, "bdin": bd,
        })
    return in_maps


def kernel(**inputs) -> np.ndarray:
    if "nc" not in _NC_CACHE:
        _NC_CACHE["nc"] = _build()
    nc = _NC_CACHE["nc"]
    in_maps = _prep_in_maps(inputs)
    global _last_in_maps
    _last_in_maps = in_maps
    res = bass_utils.run_bass_kernel_spmd(nc, in_maps,
                                          core_ids=list(range(NCORES)))
    return res.results[0]["out_loss"].reshape(2).astype(np.float32)
